# revision 17
# baseline (speedup 1.0000x reference)
"""GrokkingSNN forward on 8 TRN2 NeuronCores.

Math
----
Reference per-element recurrence (thr=1 after clamps, beta1=beta2=beta):
    m_t = beta*m_{t-1} + c - s_{t-1},  s_t = H(m_t - 1),  m_0 = 0
With q = 1 - c/((1-beta)*thr) and z = (m - m*)/thr the dynamics are
    z_1 = beta*(q-1),  z_{t+1} = beta*z_t - [z_t > q]
and the output-layer membrane is
    out = W2 @ S + G*b2,   S = sum_t beta^(15-t) s_t
        = W2 @ (beta^16*(q-1) - beta*z15 + s15) + G*b2     (telescoped)

x has only 97*97 = 9409 distinct rows and q depends only on the weights,
so the q-grid [512, 9409] is precomputed on host; the device runs the
15-step spike recurrence exactly in fp32 using three custom fused DVE
ops (2 LIF steps per instruction -> 8 vector instructions total), then
one fp16 matmul against W2 per output chunk.  The final row gather by
pair id and the G*b2 - beta^16*sum(W2) bias happen on host.
"""

import os
import sys

import numpy as np

for _p in ("/opt/trn_rl_repo",):
    if _p not in sys.path and os.path.isdir(_p):
        sys.path.insert(0, _p)

P = 97          # vocab / output dim
H = 512         # hidden
NSTEPS = 15
NCORES = 8
NGRID = P * P   # 9409 distinct input pairs
NPAD = 9472     # 8 * 1184
NLOC = NPAD // NCORES   # 1184 grid columns per core
HT = H // 128   # 4 hidden tiles
# column chunks (each <= 512 = one PSUM bank); small first chunk so compute
# starts as soon as its Q slice lands, smallish last chunk for a short tail.
# NOTE: this exact split also determines SBUF tile offsets; the (128,...,32)
# permutation measured +21% per DVE op (z/q read-port bank conflicts) — keep
# this layout unless re-measuring.
CHUNKS = [(0, 32), (32, 512), (544, 512), (1056, 128)]

_CACHE = {}
_OPS = None


# ---------------------------------------------------------------- custom ops
def _get_lif_ops():
    """Register (idempotently) the fused LIF custom DVE ops.

    LIF_INIT3_ANT: q -> z3          (init + 2 steps, 8 ALU nodes)
    LIF_STEP2_ANT: z -> z+2 steps   (6 nodes)
    LIF_FIN_ANT:   z15 -> S + beta^16*q   (5 nodes; the -beta^16 constant
                   is folded into the host-side output bias)
    """
    global _OPS
    if _OPS is not None:
        return _OPS

    from concourse.dve_ops import (
        OPS,
        CUSTOM_DVE_SPECS,
        _SUB_OPCODE_FOR_NAME,
        _CUSTOM_DVE_ROW_BASE,
        DveOp,
    )
    from concourse.dve_spec import Spec, Src0, Src1, C0, C1, lower, _has_src1
    from concourse.dve_uop import DveOpSpec

    def ref_init3(in0, in1, s0, s1, imm2):
        q = in0.astype(np.float32)
        b = np.float32(s0)
        z = q * b - b
        for _ in range(2):
            z = z * b - (z > q).astype(np.float32)
        return z

    def ref_step2(in0, in1, s0, s1, imm2):
        z = in0.astype(np.float32)
        q = in1.astype(np.float32)
        b = np.float32(s0)
        for _ in range(2):
            z = z * b - (z > q).astype(np.float32)
        return z

    def ref_fin(in0, in1, s0, s1, imm2):
        z = in0.astype(np.float32)
        q = in1.astype(np.float32)
        return z * np.float32(s0) + (z > q).astype(np.float32) + q * np.float32(s1)

    z1 = Src0 * C0 - C0
    z2 = z1 * C0 - (z1 > Src0)
    spec_init3 = Spec(body=z2 * C0 - (z2 > Src0), reference=ref_init3)

    za = Src0 * C0 - (Src0 > Src1)
    spec_step2 = Spec(body=za * C0 - (za > Src1), reference=ref_step2)

    spec_fin = Spec(
        body=Src0 * C0 + (Src0 > Src1) + Src1 * C1, reference=ref_fin
    )

    def register(name, spec):
        for op in OPS:
            if op.name == name:
                return op
        row = _CUSTOM_DVE_ROW_BASE + len(OPS)
        assert row < 0x20
        shas = {
            ver: DveOpSpec(
                name=name, opcode=row, uops=lower(spec, ver=ver),
                rd1_en=_has_src1(spec),
            ).sha(ver)
            for ver in ("v3", "v4")
        }
        op = DveOp(name, spec, subdim=False, uops_sha=shas)
        OPS.append(op)
        _SUB_OPCODE_FOR_NAME[name] = row
        CUSTOM_DVE_SPECS[name] = spec
        return op

    _OPS = (
        register("LIF_INIT3_ANT", spec_init3),
        register("LIF_STEP2_ANT", spec_step2),
        register("LIF_FIN_ANT", spec_fin),
    )
    return _OPS


# ------------------------------------------------------------------- device
def _build_bass(beta: float):
    from concourse import bacc, mybir
    from concourse.tile import TileContext

    lif_init3, lif_step2, lif_fin = _get_lif_ops()

    f32 = mybir.dt.float32
    f16 = mybir.dt.float16

    nc = bacc.Bacc("TRN2", target_bir_lowering=False, debug=False,
                   num_devices=NCORES)

    dQ = [nc.dram_tensor(f"Q{n}", (128, HT * cw), f32, kind="ExternalInput")
          for n, (c0, cw) in enumerate(CHUNKS)]
    dW2T = nc.dram_tensor("W2Tm", (128, HT * P), f16, kind="ExternalInput")
    dOUT = [nc.dram_tensor(f"out{n}", (P, cw), f16, kind="ExternalOutput")
            for n, (c0, cw) in enumerate(CHUNKS)]
    # partition ranges for the output DMAs: one DMA instruction per range
    # spreads the transfer across SDMA engines (a single [97, :] DMA lands
    # on one engine at ~25 GB/s); issues alternate sync/scalar HWDGE queues
    # so the per-instruction issue slices (~0.7us) run on two queues
    def out_splits(cw):
        if cw <= 64:
            return [(0, 97)]
        if cw <= 128:
            return [(0, 48), (48, 97)]
        return [(0, 25), (25, 50), (50, 74), (74, 97)]

    with TileContext(nc) as tc:
        with tc.tile_pool(name="const", bufs=1) as cpool, \
             tc.tile_pool(name="work", bufs=1) as wpool, \
             tc.tile_pool(name="ps", bufs=1, space="PSUM") as pspool:

            # issue Q DMAs first (compute-critical), W2 after.  Tiles are
            # created in chunk order (keeps SBUF offsets identical) but the
            # big chunk-1 DMA is ISSUED first: its ~3us transfer then
            # overlaps chunk 0's compute instead of stalling the DVE chain.
            qt = []
            for n, (c0, cw) in enumerate(CHUNKS):
                fd = HT * cw
                q = wpool.tile([128, fd], f32, tag=f"q{n}", name=f"q{n}")
                qt.append(q)
            for n in (1, 0, 2, 3):
                nc.sync.dma_start(out=qt[n], in_=dQ[n].ap())
            tW2 = cpool.tile([128, HT * P], f16, tag="tW2", name="tW2")
            nc.scalar.dma_start(out=tW2, in_=dW2T.ap())

            for n, (c0, cw) in enumerate(CHUNKS):
                fd = HT * cw
                q = qt[n]
                z = wpool.tile([128, fd], f32, tag=f"z{n}", name=f"z{n}")
                s = wpool.tile([128, fd], f16, tag=f"s{n}", name=f"s{n}")

                nc.vector._custom_dve(lif_init3, out=z, in0=q, s0=beta)
                for _ in range((NSTEPS - 3) // 2):
                    nc.vector._custom_dve(lif_step2, out=z, in0=z, in1=q,
                                          s0=beta)
                nc.vector._custom_dve(lif_fin, out=s, in0=z, in1=q,
                                      s0=-beta, s1=float(beta ** 16))

                ps = pspool.tile([P, cw], f32, tag=f"ps{n}", name=f"ps{n}")
                for i in range(HT):
                    nc.tensor.matmul(ps, tW2[:, i * P:(i + 1) * P],
                                     s[:, i * cw:(i + 1) * cw],
                                     start=(i == 0), stop=(i == HT - 1))
                ob = wpool.tile([P, cw], f16, tag=f"ob{n}", name=f"ob{n}")
                nc.scalar.copy(out=ob, in_=ps)
                for k, (p0, p1) in enumerate(out_splits(cw)):
                    eng = nc.sync if k % 2 == 0 else nc.scalar
                    eng.dma_start(out=dOUT[n].ap()[p0:p1, :],
                                  in_=ob[p0:p1, :])

    if not nc.is_finalized():
        nc.finalize()
    return nc


# -------------------------------------------------------------------- host
def _prep_inputs(x, embed_w, W1, b1, W2, b2, beta, thr):
    E = embed_w.astype(np.float64)
    W1d = W1.astype(np.float64)
    A1T = E @ W1d[:, :H].T          # [97, 512]
    A2T = E @ W1d[:, H:].T

    kq = -1.0 / ((1.0 - beta) * thr)
    qb = 1.0 + kq * b1.astype(np.float64)

    pid = np.arange(NGRID)
    v0 = pid // P
    v1 = pid % P
    # q grid, feature-major [512, 9409] -> padded [512, 9472]
    Q = np.full((H, NPAD), 100.0, np.float64)
    Q[:, :NGRID] = (A1T[v0].T + A2T[v1].T) * kq + qb[:, None]
    Q = Q.astype(np.float32)

    # W2 tiles merged along free dim: W2Tm[p, i*97+j] = W2[j, i*128+p]
    W2T = W2.T.astype(np.float16).reshape(HT, 128, P)       # [4, 128, 97]
    W2Tm = np.ascontiguousarray(W2T.transpose(1, 0, 2).reshape(128, HT * P))

    in_maps = []
    for k in range(NCORES):
        Qc = Q[:, k * NLOC:(k + 1) * NLOC]                  # [512, 1184]
        m = {"W2Tm": W2Tm}
        for n, (c0, cw) in enumerate(CHUNKS):
            # merged free-dim layout: [tile0 cols | tile1 | tile2 | tile3]
            blk = Qc[:, c0:c0 + cw].reshape(HT, 128, cw)
            m[f"Q{n}"] = np.ascontiguousarray(
                blk.transpose(1, 0, 2).reshape(128, HT * cw))
        in_maps.append(m)
    return in_maps


def kernel(x, embed_w, W1, b1, W2, b2, beta1, beta2, thr1, thr2, **_):
    from concourse.bass_utils import run_bass_kernel_spmd

    beta = float(np.clip(np.float32(beta1), 0.1, 0.9))
    beta2c = float(np.clip(np.float32(beta2), 0.1, 0.9))
    thr = float(max(np.float32(thr1), 0.1))
    assert abs(beta - beta2c) < 1e-12, "kernel assumes beta1 == beta2"

    key = round(beta, 9)
    if key not in _CACHE:
        _CACHE[key] = _build_bass(beta)
    nc = _CACHE[key]

    in_maps = _prep_inputs(x, embed_w, W1, b1, W2, b2, beta, thr)
    res = run_bass_kernel_spmd(nc, in_maps, core_ids=list(range(NCORES)))
    T = np.concatenate(
        [r[f"out{n}"] for r in res.results for n in range(len(CHUNKS))],
        axis=1)[:, :NGRID]

    G = (1.0 - beta ** NSTEPS) / (1.0 - beta)
    b2e = (G * b2.astype(np.float64)
           - (beta ** 16) * W2.astype(np.float64).sum(axis=1)).astype(np.float32)

    pid = x[:, 0].astype(np.int64) * P + x[:, 1].astype(np.int64)
    return np.ascontiguousarray(T.T[pid] + b2e[None, :]).astype(np.float32)


# revision 18
# speedup vs baseline: 1.0482x; 1.0482x over previous
"""GrokkingSNN forward on 8 TRN2 NeuronCores.

Math
----
Reference per-element recurrence (thr=1 after clamps, beta1=beta2=beta):
    m_t = beta*m_{t-1} + c - s_{t-1},  s_t = H(m_t - 1),  m_0 = 0
With q = 1 - c/((1-beta)*thr) and z = (m - m*)/thr the dynamics are
    z_1 = beta*(q-1),  z_{t+1} = beta*z_t - [z_t > q]
and the output-layer membrane is
    out = W2 @ S + G*b2,   S = sum_t beta^(15-t) s_t
        = W2 @ (beta^16*(q-1) - beta*z15 + s15) + G*b2     (telescoped)

x has only 97*97 = 9409 distinct rows and q depends only on the weights,
so the q-grid [512, 9409] is precomputed on host; the device runs the
15-step spike recurrence exactly in fp32 using three custom fused DVE
ops (2 LIF steps per instruction -> 8 vector instructions total), then
one fp16 matmul against W2 per output chunk.  The final row gather by
pair id and the G*b2 - beta^16*sum(W2) bias happen on host.
"""

import os
import sys

import numpy as np

for _p in ("/opt/trn_rl_repo",):
    if _p not in sys.path and os.path.isdir(_p):
        sys.path.insert(0, _p)

P = 97          # vocab / output dim
H = 512         # hidden
NSTEPS = 15
NCORES = 8
NGRID = P * P   # 9409 distinct input pairs
NPAD = 9472     # 8 * 1184
NLOC = NPAD // NCORES   # 1184 grid columns per core
HT = H // 128   # 4 hidden tiles
# column chunks (each <= 512 = one PSUM bank); small first chunk so compute
# starts as soon as its Q slice lands, smallish last chunk for a short tail.
# NOTE: this exact split also determines SBUF tile offsets; the (128,...,32)
# permutation measured +21% per DVE op (z/q read-port bank conflicts) — keep
# this layout unless re-measuring.
CHUNKS = [(0, 32), (32, 512), (544, 512), (1056, 128)]

_CACHE = {}
_OPS = None


# ---------------------------------------------------------------- custom ops
def _get_lif_ops():
    """Register (idempotently) the fused LIF custom DVE ops.

    LIF_INIT3_ANT: q -> z3          (init + 2 steps, 8 ALU nodes)
    LIF_STEP2_ANT: z -> z+2 steps   (6 nodes)
    LIF_FIN_ANT:   z15 -> S + beta^16*q   (5 nodes; the -beta^16 constant
                   is folded into the host-side output bias)
    """
    global _OPS
    if _OPS is not None:
        return _OPS

    from concourse.dve_ops import (
        OPS,
        CUSTOM_DVE_SPECS,
        _SUB_OPCODE_FOR_NAME,
        _CUSTOM_DVE_ROW_BASE,
        DveOp,
    )
    from concourse.dve_spec import Spec, Src0, Src1, C0, C1, lower, _has_src1
    from concourse.dve_uop import DveOpSpec

    def ref_init3(in0, in1, s0, s1, imm2):
        q = in0.astype(np.float32)
        b = np.float32(s0)
        z = q * b - b
        for _ in range(2):
            z = z * b - (z > q).astype(np.float32)
        return z

    def ref_step2(in0, in1, s0, s1, imm2):
        z = in0.astype(np.float32)
        q = in1.astype(np.float32)
        b = np.float32(s0)
        for _ in range(2):
            z = z * b - (z > q).astype(np.float32)
        return z

    def ref_fin(in0, in1, s0, s1, imm2):
        z = in0.astype(np.float32)
        q = in1.astype(np.float32)
        return z * np.float32(s0) + (z > q).astype(np.float32) + q * np.float32(s1)

    z1 = Src0 * C0 - C0
    z2 = z1 * C0 - (z1 > Src0)
    spec_init3 = Spec(body=z2 * C0 - (z2 > Src0), reference=ref_init3)

    za = Src0 * C0 - (Src0 > Src1)
    spec_step2 = Spec(body=za * C0 - (za > Src1), reference=ref_step2)

    spec_fin = Spec(
        body=Src0 * C0 + (Src0 > Src1) + Src1 * C1, reference=ref_fin
    )

    def register(name, spec):
        for op in OPS:
            if op.name == name:
                return op
        row = _CUSTOM_DVE_ROW_BASE + len(OPS)
        assert row < 0x20
        shas = {
            ver: DveOpSpec(
                name=name, opcode=row, uops=lower(spec, ver=ver),
                rd1_en=_has_src1(spec),
            ).sha(ver)
            for ver in ("v3", "v4")
        }
        op = DveOp(name, spec, subdim=False, uops_sha=shas)
        OPS.append(op)
        _SUB_OPCODE_FOR_NAME[name] = row
        CUSTOM_DVE_SPECS[name] = spec
        return op

    _OPS = (
        register("LIF_INIT3_ANT", spec_init3),
        register("LIF_STEP2_ANT", spec_step2),
        register("LIF_FIN_ANT", spec_fin),
    )
    return _OPS


# ------------------------------------------------------------------- device
def _build_bass(beta: float):
    from concourse import bacc, mybir
    from concourse.tile import TileContext

    lif_init3, lif_step2, lif_fin = _get_lif_ops()

    f32 = mybir.dt.float32
    f16 = mybir.dt.float16

    nc = bacc.Bacc("TRN2", target_bir_lowering=False, debug=False,
                   num_devices=NCORES)

    dQ = [nc.dram_tensor(f"Q{n}", (128, HT * cw), f32, kind="ExternalInput")
          for n, (c0, cw) in enumerate(CHUNKS)]
    dW2T = nc.dram_tensor("W2Tm", (128, HT * P), f16, kind="ExternalInput")
    dOUT = [nc.dram_tensor(f"out{n}", (P, cw), f16, kind="ExternalOutput")
            for n, (c0, cw) in enumerate(CHUNKS)]
    # partition ranges for the output DMAs: one DMA instruction per range
    # spreads the transfer across SDMA engines (a single [97, :] DMA lands
    # on one engine at ~25 GB/s); issues alternate sync/scalar HWDGE queues
    # so the per-instruction issue slices (~0.7us) run on two queues
    def out_splits(cw):
        if cw <= 64:
            return [(0, 97)]
        if cw <= 128:
            return [(0, 48), (48, 97)]
        return [(0, 25), (25, 50), (50, 74), (74, 97)]

    with TileContext(nc) as tc:
        with tc.tile_pool(name="const", bufs=1) as cpool, \
             tc.tile_pool(name="work", bufs=1) as wpool, \
             tc.tile_pool(name="ps", bufs=1, space="PSUM") as pspool:

            # issue Q DMAs first (compute-critical), W2 after.  Chunk-0
            # first: the tile scheduler orders the DVE chain by data
            # readiness, so the small chunk's quick DMA gives the chain a
            # ~3.4us head start (issuing the big chunk first was measured
            # 4us WORSE — the whole pipeline reorders behind it).
            qt = []
            for n, (c0, cw) in enumerate(CHUNKS):
                fd = HT * cw
                q = wpool.tile([128, fd], f32, tag=f"q{n}", name=f"q{n}")
                nc.sync.dma_start(out=q, in_=dQ[n].ap())
                qt.append(q)
            tW2 = cpool.tile([128, HT * P], f16, tag="tW2", name="tW2")
            nc.scalar.dma_start(out=tW2, in_=dW2T.ap())

            for n, (c0, cw) in enumerate(CHUNKS):
                fd = HT * cw
                q = qt[n]
                z = wpool.tile([128, fd], f32, tag=f"z{n}", name=f"z{n}")
                s = wpool.tile([128, fd], f16, tag=f"s{n}", name=f"s{n}")

                nc.vector._custom_dve(lif_init3, out=z, in0=q, s0=beta)
                for _ in range((NSTEPS - 3) // 2):
                    nc.vector._custom_dve(lif_step2, out=z, in0=z, in1=q,
                                          s0=beta)
                nc.vector._custom_dve(lif_fin, out=s, in0=z, in1=q,
                                      s0=-beta, s1=float(beta ** 16))

                ps = pspool.tile([P, cw], f32, tag=f"ps{n}", name=f"ps{n}")
                for i in range(HT):
                    nc.tensor.matmul(ps, tW2[:, i * P:(i + 1) * P],
                                     s[:, i * cw:(i + 1) * cw],
                                     start=(i == 0), stop=(i == HT - 1))
                ob = wpool.tile([P, cw], f16, tag=f"ob{n}", name=f"ob{n}")
                nc.scalar.copy(out=ob, in_=ps)
                for k, (p0, p1) in enumerate(out_splits(cw)):
                    eng = nc.sync if k % 2 == 0 else nc.scalar
                    eng.dma_start(out=dOUT[n].ap()[p0:p1, :],
                                  in_=ob[p0:p1, :])

    if not nc.is_finalized():
        nc.finalize()
    return nc


# -------------------------------------------------------------------- host
def _prep_inputs(x, embed_w, W1, b1, W2, b2, beta, thr):
    E = embed_w.astype(np.float64)
    W1d = W1.astype(np.float64)
    A1T = E @ W1d[:, :H].T          # [97, 512]
    A2T = E @ W1d[:, H:].T

    kq = -1.0 / ((1.0 - beta) * thr)
    qb = 1.0 + kq * b1.astype(np.float64)

    pid = np.arange(NGRID)
    v0 = pid // P
    v1 = pid % P
    # q grid, feature-major [512, 9409] -> padded [512, 9472]
    Q = np.full((H, NPAD), 100.0, np.float64)
    Q[:, :NGRID] = (A1T[v0].T + A2T[v1].T) * kq + qb[:, None]
    Q = Q.astype(np.float32)

    # W2 tiles merged along free dim: W2Tm[p, i*97+j] = W2[j, i*128+p]
    W2T = W2.T.astype(np.float16).reshape(HT, 128, P)       # [4, 128, 97]
    W2Tm = np.ascontiguousarray(W2T.transpose(1, 0, 2).reshape(128, HT * P))

    in_maps = []
    for k in range(NCORES):
        Qc = Q[:, k * NLOC:(k + 1) * NLOC]                  # [512, 1184]
        m = {"W2Tm": W2Tm}
        for n, (c0, cw) in enumerate(CHUNKS):
            # merged free-dim layout: [tile0 cols | tile1 | tile2 | tile3]
            blk = Qc[:, c0:c0 + cw].reshape(HT, 128, cw)
            m[f"Q{n}"] = np.ascontiguousarray(
                blk.transpose(1, 0, 2).reshape(128, HT * cw))
        in_maps.append(m)
    return in_maps


def kernel(x, embed_w, W1, b1, W2, b2, beta1, beta2, thr1, thr2, **_):
    from concourse.bass_utils import run_bass_kernel_spmd

    beta = float(np.clip(np.float32(beta1), 0.1, 0.9))
    beta2c = float(np.clip(np.float32(beta2), 0.1, 0.9))
    thr = float(max(np.float32(thr1), 0.1))
    assert abs(beta - beta2c) < 1e-12, "kernel assumes beta1 == beta2"

    key = round(beta, 9)
    if key not in _CACHE:
        _CACHE[key] = _build_bass(beta)
    nc = _CACHE[key]

    in_maps = _prep_inputs(x, embed_w, W1, b1, W2, b2, beta, thr)
    res = run_bass_kernel_spmd(nc, in_maps, core_ids=list(range(NCORES)))
    T = np.concatenate(
        [r[f"out{n}"] for r in res.results for n in range(len(CHUNKS))],
        axis=1)[:, :NGRID]

    G = (1.0 - beta ** NSTEPS) / (1.0 - beta)
    b2e = (G * b2.astype(np.float64)
           - (beta ** 16) * W2.astype(np.float64).sum(axis=1)).astype(np.float32)

    pid = x[:, 0].astype(np.int64) * P + x[:, 1].astype(np.int64)
    return np.ascontiguousarray(T.T[pid] + b2e[None, :]).astype(np.float32)


# revision 19
# speedup vs baseline: 1.0551x; 1.0065x over previous
"""GrokkingSNN forward on 8 TRN2 NeuronCores.

Math
----
Reference per-element recurrence (thr=1 after clamps, beta1=beta2=beta):
    m_t = beta*m_{t-1} + c - s_{t-1},  s_t = H(m_t - 1),  m_0 = 0
With q = 1 - c/((1-beta)*thr) and z = (m - m*)/thr the dynamics are
    z_1 = beta*(q-1),  z_{t+1} = beta*z_t - [z_t > q]
and the output-layer membrane is
    out = W2 @ S + G*b2,   S = sum_t beta^(15-t) s_t
        = W2 @ (beta^16*(q-1) - beta*z15 + s15) + G*b2     (telescoped)

x has only 97*97 = 9409 distinct rows and q depends only on the weights,
so the q-grid [512, 9409] is precomputed on host; the device runs the
15-step spike recurrence exactly in fp32 using three custom fused DVE
ops (2 LIF steps per instruction -> 8 vector instructions total), then
one fp16 matmul against W2 per output chunk.  The final row gather by
pair id and the G*b2 - beta^16*sum(W2) bias happen on host.
"""

import os
import sys

import numpy as np

for _p in ("/opt/trn_rl_repo",):
    if _p not in sys.path and os.path.isdir(_p):
        sys.path.insert(0, _p)

P = 97          # vocab / output dim
H = 512         # hidden
NSTEPS = 15
NCORES = 8
NGRID = P * P   # 9409 distinct input pairs
NPAD = 9472     # 8 * 1184
NLOC = NPAD // NCORES   # 1184 grid columns per core
HT = H // 128   # 4 hidden tiles
# column chunks (each <= 512 = one PSUM bank); small first chunk so compute
# starts as soon as its Q slice lands, smallish last chunk for a short tail.
# NOTE: this exact split also determines SBUF tile offsets; the (128,...,32)
# permutation measured +21% per DVE op (z/q read-port bank conflicts) — keep
# this layout unless re-measuring.
CHUNKS = [(0, 32), (32, 512), (544, 512), (1056, 128)]

_CACHE = {}
_OPS = None


# ---------------------------------------------------------------- custom ops
def _get_lif_ops():
    """Register (idempotently) the fused LIF custom DVE ops.

    LIF_INIT3_ANT: q -> z3          (init + 2 steps, 8 ALU nodes)
    LIF_STEP2_ANT: z -> z+2 steps   (6 nodes)
    LIF_FIN_ANT:   z15 -> S + beta^16*q   (5 nodes; the -beta^16 constant
                   is folded into the host-side output bias)
    """
    global _OPS
    if _OPS is not None:
        return _OPS

    from concourse.dve_ops import (
        OPS,
        CUSTOM_DVE_SPECS,
        _SUB_OPCODE_FOR_NAME,
        _CUSTOM_DVE_ROW_BASE,
        DveOp,
    )
    from concourse.dve_spec import Spec, Src0, Src1, C0, C1, lower, _has_src1
    from concourse.dve_uop import DveOpSpec

    def ref_init3(in0, in1, s0, s1, imm2):
        q = in0.astype(np.float32)
        b = np.float32(s0)
        z = q * b - b
        for _ in range(2):
            z = z * b - (z > q).astype(np.float32)
        return z

    def ref_step2(in0, in1, s0, s1, imm2):
        z = in0.astype(np.float32)
        q = in1.astype(np.float32)
        b = np.float32(s0)
        for _ in range(2):
            z = z * b - (z > q).astype(np.float32)
        return z

    def ref_fin(in0, in1, s0, s1, imm2):
        z = in0.astype(np.float32)
        q = in1.astype(np.float32)
        return z * np.float32(s0) + (z > q).astype(np.float32) + q * np.float32(s1)

    z1 = Src0 * C0 - C0
    z2 = z1 * C0 - (z1 > Src0)
    spec_init3 = Spec(body=z2 * C0 - (z2 > Src0), reference=ref_init3)

    za = Src0 * C0 - (Src0 > Src1)
    spec_step2 = Spec(body=za * C0 - (za > Src1), reference=ref_step2)

    spec_fin = Spec(
        body=Src0 * C0 + (Src0 > Src1) + Src1 * C1, reference=ref_fin
    )

    def register(name, spec):
        for op in OPS:
            if op.name == name:
                return op
        row = _CUSTOM_DVE_ROW_BASE + len(OPS)
        assert row < 0x20
        shas = {
            ver: DveOpSpec(
                name=name, opcode=row, uops=lower(spec, ver=ver),
                rd1_en=_has_src1(spec),
            ).sha(ver)
            for ver in ("v3", "v4")
        }
        op = DveOp(name, spec, subdim=False, uops_sha=shas)
        OPS.append(op)
        _SUB_OPCODE_FOR_NAME[name] = row
        CUSTOM_DVE_SPECS[name] = spec
        return op

    _OPS = (
        register("LIF_INIT3_ANT", spec_init3),
        register("LIF_STEP2_ANT", spec_step2),
        register("LIF_FIN_ANT", spec_fin),
    )
    return _OPS


# ------------------------------------------------------------------- device
def _build_bass(beta: float):
    from concourse import bacc, mybir
    from concourse.tile import TileContext

    lif_init3, lif_step2, lif_fin = _get_lif_ops()

    f32 = mybir.dt.float32
    f16 = mybir.dt.float16

    nc = bacc.Bacc("TRN2", target_bir_lowering=False, debug=False,
                   num_devices=NCORES)

    dQ = [nc.dram_tensor(f"Q{n}", (128, HT * cw), f32, kind="ExternalInput")
          for n, (c0, cw) in enumerate(CHUNKS)]
    dW2T = nc.dram_tensor("W2Tm", (128, HT * P), f16, kind="ExternalInput")
    dOUT = [nc.dram_tensor(f"out{n}", (P, cw), f16, kind="ExternalOutput")
            for n, (c0, cw) in enumerate(CHUNKS)]
    # partition ranges for the output DMAs: one DMA instruction per range
    # spreads the transfer across SDMA engines (a single [97, :] DMA lands
    # on one engine at ~25 GB/s); issues alternate sync/scalar HWDGE queues
    # so the per-instruction issue slices (~0.7us) run on two queues
    def out_splits(cw):
        if cw <= 64:
            return [(0, 97)]
        if cw <= 128:
            return [(0, 48), (48, 97)]
        return [(0, 25), (25, 50), (50, 74), (74, 97)]

    with TileContext(nc) as tc:
        with tc.tile_pool(name="const", bufs=1) as cpool, \
             tc.tile_pool(name="work", bufs=1) as wpool, \
             tc.tile_pool(name="ps", bufs=1, space="PSUM") as pspool:

            # issue Q DMAs first (compute-critical), W2 after.  Chunk-0
            # first: the tile scheduler orders the DVE chain by data
            # readiness, so the small chunk's quick DMA gives the chain a
            # ~3.4us head start (issuing the big chunk first was measured
            # 4us WORSE — the whole pipeline reorders behind it).  Q1 goes
            # on the scalar HWDGE queue so its ~3us transfer starts in
            # parallel with Q0's instead of behind it (closes a ~1.2us
            # DVE stall between chunks 0 and 1); Q0 still lands first so
            # the chain order is unchanged.
            qt = []
            for n, (c0, cw) in enumerate(CHUNKS):
                fd = HT * cw
                q = wpool.tile([128, fd], f32, tag=f"q{n}", name=f"q{n}")
                eng = nc.scalar if n == 1 else nc.sync
                eng.dma_start(out=q, in_=dQ[n].ap())
                qt.append(q)
            tW2 = cpool.tile([128, HT * P], f16, tag="tW2", name="tW2")
            nc.scalar.dma_start(out=tW2, in_=dW2T.ap())

            for n, (c0, cw) in enumerate(CHUNKS):
                fd = HT * cw
                q = qt[n]
                z = wpool.tile([128, fd], f32, tag=f"z{n}", name=f"z{n}")
                s = wpool.tile([128, fd], f16, tag=f"s{n}", name=f"s{n}")

                nc.vector._custom_dve(lif_init3, out=z, in0=q, s0=beta)
                for _ in range((NSTEPS - 3) // 2):
                    nc.vector._custom_dve(lif_step2, out=z, in0=z, in1=q,
                                          s0=beta)
                nc.vector._custom_dve(lif_fin, out=s, in0=z, in1=q,
                                      s0=-beta, s1=float(beta ** 16))

                ps = pspool.tile([P, cw], f32, tag=f"ps{n}", name=f"ps{n}")
                for i in range(HT):
                    nc.tensor.matmul(ps, tW2[:, i * P:(i + 1) * P],
                                     s[:, i * cw:(i + 1) * cw],
                                     start=(i == 0), stop=(i == HT - 1))
                ob = wpool.tile([P, cw], f16, tag=f"ob{n}", name=f"ob{n}")
                nc.scalar.copy(out=ob, in_=ps)
                for k, (p0, p1) in enumerate(out_splits(cw)):
                    eng = nc.sync if k % 2 == 0 else nc.scalar
                    eng.dma_start(out=dOUT[n].ap()[p0:p1, :],
                                  in_=ob[p0:p1, :])

    if not nc.is_finalized():
        nc.finalize()
    return nc


# -------------------------------------------------------------------- host
def _prep_inputs(x, embed_w, W1, b1, W2, b2, beta, thr):
    E = embed_w.astype(np.float64)
    W1d = W1.astype(np.float64)
    A1T = E @ W1d[:, :H].T          # [97, 512]
    A2T = E @ W1d[:, H:].T

    kq = -1.0 / ((1.0 - beta) * thr)
    qb = 1.0 + kq * b1.astype(np.float64)

    pid = np.arange(NGRID)
    v0 = pid // P
    v1 = pid % P
    # q grid, feature-major [512, 9409] -> padded [512, 9472]
    Q = np.full((H, NPAD), 100.0, np.float64)
    Q[:, :NGRID] = (A1T[v0].T + A2T[v1].T) * kq + qb[:, None]
    Q = Q.astype(np.float32)

    # W2 tiles merged along free dim: W2Tm[p, i*97+j] = W2[j, i*128+p]
    W2T = W2.T.astype(np.float16).reshape(HT, 128, P)       # [4, 128, 97]
    W2Tm = np.ascontiguousarray(W2T.transpose(1, 0, 2).reshape(128, HT * P))

    in_maps = []
    for k in range(NCORES):
        Qc = Q[:, k * NLOC:(k + 1) * NLOC]                  # [512, 1184]
        m = {"W2Tm": W2Tm}
        for n, (c0, cw) in enumerate(CHUNKS):
            # merged free-dim layout: [tile0 cols | tile1 | tile2 | tile3]
            blk = Qc[:, c0:c0 + cw].reshape(HT, 128, cw)
            m[f"Q{n}"] = np.ascontiguousarray(
                blk.transpose(1, 0, 2).reshape(128, HT * cw))
        in_maps.append(m)
    return in_maps


def kernel(x, embed_w, W1, b1, W2, b2, beta1, beta2, thr1, thr2, **_):
    from concourse.bass_utils import run_bass_kernel_spmd

    beta = float(np.clip(np.float32(beta1), 0.1, 0.9))
    beta2c = float(np.clip(np.float32(beta2), 0.1, 0.9))
    thr = float(max(np.float32(thr1), 0.1))
    assert abs(beta - beta2c) < 1e-12, "kernel assumes beta1 == beta2"

    key = round(beta, 9)
    if key not in _CACHE:
        _CACHE[key] = _build_bass(beta)
    nc = _CACHE[key]

    in_maps = _prep_inputs(x, embed_w, W1, b1, W2, b2, beta, thr)
    res = run_bass_kernel_spmd(nc, in_maps, core_ids=list(range(NCORES)))
    T = np.concatenate(
        [r[f"out{n}"] for r in res.results for n in range(len(CHUNKS))],
        axis=1)[:, :NGRID]

    G = (1.0 - beta ** NSTEPS) / (1.0 - beta)
    b2e = (G * b2.astype(np.float64)
           - (beta ** 16) * W2.astype(np.float64).sum(axis=1)).astype(np.float32)

    pid = x[:, 0].astype(np.int64) * P + x[:, 1].astype(np.int64)
    return np.ascontiguousarray(T.T[pid] + b2e[None, :]).astype(np.float32)
